# revision 1
# baseline (speedup 1.0000x reference)
"""Bass/Trainium2 kernel for nn_LocalAttention (banded attention, window 16).

Self-contained: takes full inputs, shards over 8 NeuronCores as
(batch, head-octet, seq-half), runs a banded-attention Bass kernel per core,
gathers on host.

Math: the reference zeroes out-of-band scores (not -inf) and softmaxes the
FULL row, so out-of-band entries contribute exp(0)=1.  With
em1 = band_mask_applied(exp(s)) - 1 (exactly 0 off-band and on padded keys):
  Z_i   = sum_window(em1) + S
  num_i = sum_window(em1 * v) + sum_all(v)
so only a 144-wide banded computation per 128-query block is needed.
Scores are computed transposed ([keys, queries]) so em1 feeds the ctx matmul
as lhsT directly (no transposes); Z comes from an ones-column in V; 1/Z is
broadcast across partitions with a rank-1 matmul.  Biases bq/bk enter via an
augmented ones-row of x (zero on padded keys, so padding stays exact), and
bv/bo are folded on the host (softmax rows sum to 1).
"""
import os
import sys

for _p in ("/opt/trn_rl_repo",):
    if os.path.isdir(_p) and _p not in sys.path:
        sys.path.append(_p)

import numpy as np
import ml_dtypes

B, S, D = 2, 2048, 1024
H, HD = 16, 64
W = 16                    # band half-width 8
SC = 1024                 # seq chunk per core
HK = SC + W               # key halo chunk (1040)
HC = 512                  # head-dim columns per core (8 heads)
NBLK = SC // 128          # query blocks per head per core (8)
NH = HC // HD             # heads per core (8)
VST = 64                  # V stride per head in vaug

_CACHE = {}


def _build():
    import concourse.bacc as bacc
    import concourse.tile as tile
    from concourse import mybir

    f32 = mybir.dt.float32
    f32r = mybir.dt.float32r
    bf16 = mybir.dt.bfloat16

    nc = bacc.Bacc("TRN2", target_bir_lowering=False, debug=False, num_devices=8)

    xt = nc.dram_tensor("xt", [D, HK], f32r, kind="ExternalInput").ap()
    xa = nc.dram_tensor("xa", [1, HK], f32r, kind="ExternalInput").ap()
    wq = nc.dram_tensor("wq", [D, HC], f32r, kind="ExternalInput").ap()
    wk = nc.dram_tensor("wk", [D, HC], f32r, kind="ExternalInput").ap()
    wv = nc.dram_tensor("wv", [D, HC], f32r, kind="ExternalInput").ap()
    wo = nc.dram_tensor("wo", [HC, D], f32r, kind="ExternalInput").ap()
    bqr = nc.dram_tensor("bqr", [1, HC], f32r, kind="ExternalInput").ap()
    bkr = nc.dram_tensor("bkr", [1, HC], f32r, kind="ExternalInput").ap()
    vsum = nc.dram_tensor("vsum", [HC], f32, kind="ExternalInput").ap()
    maskt = nc.dram_tensor("maskt", [128, 512], f32, kind="ExternalInput").ap()
    out = nc.dram_tensor("out", [SC, D], f32, kind="ExternalOutput").ap()

    KD = D // 128     # 8 contraction tiles
    Exp = mybir.ActivationFunctionType.Exp
    NVT = (HK + 127) // 128   # 9 V row tiles (last has 16 rows)

    with tile.TileContext(nc) as tc:
        with tc.tile_pool(name="stat", bufs=1) as stat, \
             tc.tile_pool(name="acts", bufs=1) as acts, \
             tc.tile_pool(name="blk", bufs=3) as blk, \
             tc.tile_pool(name="sml", bufs=4) as sml, \
             tc.tile_pool(name="ob", bufs=3) as ob, \
             tc.tile_pool(name="pmm", bufs=3, space="PSUM") as pmm, \
             tc.tile_pool(name="pst", bufs=2, space="PSUM") as pst, \
             tc.tile_pool(name="pcc", bufs=2, space="PSUM") as pcc, \
             tc.tile_pool(name="pzb", bufs=1, space="PSUM") as pzb:

            # ---- static inputs -> SBUF (spread over both HWDGE engines) ----
            xt_sb = stat.tile([128, KD, HK], f32r)
            xt_r = xt.rearrange("(o p) f -> p o f", p=128)
            for k in range(KD):
                nc.sync.dma_start(xt_sb[:, k], xt_r[:, k])
            xa_sb = stat.tile([1, HK], f32r)
            nc.sync.dma_start(xa_sb[:], xa)
            wq_sb = stat.tile([128, KD, HC], f32r)
            wk_sb = stat.tile([128, KD, HC], f32r)
            wv_sb = stat.tile([128, KD, HC], f32r)
            for w_sb, w_dr in ((wq_sb, wq), (wk_sb, wk), (wv_sb, wv)):
                w_r = w_dr.rearrange("(o p) f -> p o f", p=128)
                for k in range(KD):
                    nc.scalar.dma_start(w_sb[:, k], w_r[:, k])
            wo_sb = stat.tile([128, HC // 128, D], f32r)
            wo_r = wo.rearrange("(o p) f -> p o f", p=128)
            for k in range(HC // 128):
                nc.scalar.dma_start(wo_sb[:, k], wo_r[:, k])
            bqr_sb = stat.tile([1, HC], f32r)
            nc.sync.dma_start(bqr_sb[:], bqr)
            bkr_sb = stat.tile([1, HC], f32r)
            nc.sync.dma_start(bkr_sb[:], bkr)
            # vsum^T per head: [64, NH], head h at column h, partitions 0:64
            vsum_pc = stat.tile([64, NH], f32)
            nc.sync.dma_start(vsum_pc[:], vsum.rearrange("(h c) -> c h", c=64))
            mask_sb = stat.tile([128, 512], f32)
            nc.sync.dma_start(mask_sb[:], maskt)

            # ---- projections ----
            qt_sb = acts.tile([128, HC // 128, SC], bf16)   # Q^T (scaled 1/8)
            kt_sb = acts.tile([128, HC // 128, HK], bf16)   # K^T over halo keys
            vaug_sb = acts.tile([128, NVT, NH * VST], bf16)  # V per head
            ctxt_sb = acts.tile([128, HC // 128, SC], f32r)  # ctx^T
            onesm_sb = stat.tile([128, 64], bf16)            # zb matmul lhsT
            nc.gpsimd.memset(onesm_sb[:], 1.0)

            # Q^T = (x @ Wq + 1 x bq)^T * 0.125
            for m in range(HC // 128):
                for nch in range(SC // 512):
                    ps = pmm.tile([128, 512], f32, tag="mm")
                    for k in range(KD):
                        nc.tensor.matmul(
                            ps[:], wq_sb[:, k, m * 128:(m + 1) * 128],
                            xt_sb[:, k, 8 + nch * 512: 8 + (nch + 1) * 512],
                            start=(k == 0), stop=False)
                    nc.tensor.matmul(
                        ps[:], bqr_sb[0:1, m * 128:(m + 1) * 128],
                        xa_sb[0:1, 8 + nch * 512: 8 + (nch + 1) * 512],
                        start=False, stop=True)
                    nc.vector.tensor_scalar_mul(
                        qt_sb[:, m, nch * 512:(nch + 1) * 512], ps[:], 0.125)

            # K^T over all HK halo keys
            k_chunks = [(0, 512), (512, 512), (1024, HK - 1024)]
            for m in range(HC // 128):
                for (c0, cw) in k_chunks:
                    ps = pmm.tile([128, 512], f32, tag="mm")
                    for k in range(KD):
                        nc.tensor.matmul(
                            ps[:, :cw], wk_sb[:, k, m * 128:(m + 1) * 128],
                            xt_sb[:, k, c0:c0 + cw],
                            start=(k == 0), stop=False)
                    nc.tensor.matmul(
                        ps[:, :cw], bkr_sb[0:1, m * 128:(m + 1) * 128],
                        xa_sb[0:1, c0:c0 + cw], start=False, stop=True)
                    nc.vector.tensor_copy(kt_sb[:, m, c0:c0 + cw], ps[:, :cw])

            # V natural [HK, HC] -> vaug (stride-65 per head, col 64 = ones)
            for mt in range(NVT):
                rows = min(128, HK - mt * 128)
                ps = pmm.tile([128, 512], f32, tag="mm")
                for k in range(KD):
                    nc.tensor.matmul(
                        ps[:rows, :HC],
                        xt_sb[:, k, mt * 128: mt * 128 + rows],
                        wv_sb[:, k, :], start=(k == 0), stop=(k == KD - 1))
                nc.vector.tensor_copy(vaug_sb[:rows, mt, :], ps[:rows, :HC])

            # ---- banded attention ----
            # scores transposed [keys, queries]; batch 2 blocks per psum/ops,
            # 4 blocks per ctx psum/epilogue.
            for h in range(NH):
                hp, hr = h // 2, (h % 2) * 64
                for tt in range(NBLK // 4):      # super-block of 4 q-blocks
                    em_list = []
                    for half in range(2):        # 2 q-blocks per scores batch
                        psT = pst.tile([128, 512], f32, tag="st")
                        for j in range(2):
                            t = tt * 4 + half * 2 + j
                            nc.tensor.matmul(
                                psT[:, j * 256: j * 256 + 128],
                                kt_sb[hr:hr + 64, hp, t * 128: t * 128 + 128],
                                qt_sb[hr:hr + 64, hp, t * 128:(t + 1) * 128],
                                start=True, stop=True)
                            nc.tensor.matmul(
                                psT[0:16, j * 256 + 128: j * 256 + 256],
                                kt_sb[hr:hr + 64, hp,
                                      t * 128 + 128: t * 128 + 144],
                                qt_sb[hr:hr + 64, hp, t * 128:(t + 1) * 128],
                                start=True, stop=True)
                        w0 = blk.tile([128, 512], f32, tag="w0")
                        nc.vector.tensor_tensor(w0[:], psT[:], mask_sb[:],
                                                mybir.AluOpType.mult)
                        em = blk.tile([128, 512], bf16, tag="em")
                        nc.scalar.activation(out=em[:], in_=w0[:], func=Exp)
                        nc.vector.tensor_scalar_add(em[:], em[:], -1.0)
                        em_list.append(em)

                    # ctx^T num for 4 blocks into one [64, 512] psum, and
                    # Z broadcast [64, 512] via all-ones lhsT matmuls
                    ps_c = pcc.tile([64, 512], f32, tag="cc")
                    ps_z = pzb.tile([64, 512], f32, tag="zb")
                    for q in range(4):
                        t = tt * 4 + q
                        em = em_list[q // 2]
                        off = (q % 2) * 256
                        nc.tensor.matmul(
                            ps_c[:, q * 128:(q + 1) * 128],
                            vaug_sb[:, t, h * VST:(h + 1) * VST],
                            em[:, off: off + 128], start=True, stop=False)
                        nc.tensor.matmul(
                            ps_c[:, q * 128:(q + 1) * 128],
                            vaug_sb[0:16, t + 1, h * VST:(h + 1) * VST],
                            em[0:16, off + 128: off + 256],
                            start=False, stop=True)
                        nc.tensor.matmul(
                            ps_z[:, q * 128:(q + 1) * 128],
                            onesm_sb[:], em[:, off: off + 128],
                            start=True, stop=False)
                        nc.tensor.matmul(
                            ps_z[:, q * 128:(q + 1) * 128],
                            onesm_sb[0:16, :], em[0:16, off + 128: off + 256],
                            start=False, stop=True)
                    # ctx = (num + vsum) / (z + S), reciprocal via fast approx
                    zc = sml.tile([64, 512], f32, tag="zc")
                    nc.vector.tensor_scalar_add(zc[:], ps_z[:], float(S))
                    rz = sml.tile([64, 512], f32, tag="rz")
                    nc.vector.reciprocal_approx_fast(rz[:], zc[:])
                    cs = sml.tile([64, 512], f32, tag="cs")
                    nc.vector.tensor_scalar_add(cs[:], ps_c[:],
                                                vsum_pc[:, h:h + 1])
                    if hr == 0:
                        nc.vector.tensor_tensor(
                            ctxt_sb[0:64, hp, tt * 512:(tt + 1) * 512],
                            cs[:], rz[:], mybir.AluOpType.mult)
                    else:
                        cr = sml.tile([64, 512], f32r, tag="cr")
                        nc.vector.tensor_tensor(cr[:], cs[:], rz[:],
                                                mybir.AluOpType.mult)
                        nc.sync.dma_start(
                            ctxt_sb[64:128, hp, tt * 512:(tt + 1) * 512],
                            cr[:])

            # ---- out projection ----
            for st in range(SC // 128):
                for nch in range(D // 512):
                    ps = pmm.tile([128, 512], f32, tag="mm")
                    for kt in range(HC // 128):
                        nc.tensor.matmul(
                            ps[:], ctxt_sb[:, kt, st * 128:(st + 1) * 128],
                            wo_sb[:, kt, nch * 512:(nch + 1) * 512],
                            start=(kt == 0), stop=(kt == HC // 128 - 1))
                    o_sb = ob.tile([128, 512], f32)
                    nc.vector.tensor_copy(o_sb[:], ps[:])
                    nc.sync.dma_start(
                        out[st * 128:(st + 1) * 128, nch * 512:(nch + 1) * 512],
                        o_sb[:])

    nc.compile()
    return nc


def _get_nc():
    if "nc" not in _CACHE:
        _CACHE["nc"] = _build()
    return _CACHE["nc"]


LAST_EXEC_NS = None


def _band_maskt():
    """[128, 512] f32: two copies of the transposed-window mask pair."""
    m = np.zeros((128, 512), np.float32)
    r = np.arange(128)[:, None]
    c = np.arange(128)[None, :]
    main = ((c <= r) & (r <= c + W)).astype(np.float32)   # keys 0..127
    r2 = np.arange(16)[:, None]
    tail = (c >= 112 + r2).astype(np.float32)             # keys 128..143
    for j in (0, 1):
        m[:, j * 256: j * 256 + 128] = main
        m[:16, j * 256 + 128: j * 256 + 256] = tail
    return m


def kernel(hidden_states, Wq, bq, Wk, bk, Wv, bv, Wo, bo):
    global LAST_EXEC_NS
    from concourse.bass_utils import run_bass_kernel_spmd

    hs = np.asarray(hidden_states, dtype=np.float32)
    Wq, Wk, Wv, Wo = (np.asarray(a, dtype=np.float32) for a in (Wq, Wk, Wv, Wo))
    bq, bk, bv, bo = (np.asarray(a, dtype=np.float32) for a in (bq, bk, bv, bo))

    xpad = np.zeros((B, S + W, D), np.float32)
    xpad[:, 8:8 + S] = hs
    xT = np.ascontiguousarray(xpad.transpose(0, 2, 1))  # [B, D, S+W]
    xav = np.zeros((B, S + W), np.float32)
    xav[:, 8:8 + S] = 1.0                               # ones row (0 on pads)

    maskt = _band_maskt()

    in_maps = []
    for core in range(8):
        b, hg, sh = core // 4, (core // 2) % 2, core % 2
        cols = slice(hg * HC, (hg + 1) * HC)
        vs = xpad[b].sum(0, dtype=np.float64) @ Wv[:, cols].astype(np.float64)
        in_maps.append({
            "xt": np.ascontiguousarray(xT[b][:, sh * SC: sh * SC + HK]),
            "xa": np.ascontiguousarray(xav[b][None, sh * SC: sh * SC + HK]),
            "wq": np.ascontiguousarray(Wq[:, cols]),
            "wk": np.ascontiguousarray(Wk[:, cols]),
            "wv": np.ascontiguousarray(Wv[:, cols]),
            "wo": np.ascontiguousarray(Wo[cols, :]),
            "bqr": np.ascontiguousarray(bq[None, cols]),
            "bkr": np.ascontiguousarray(bk[None, cols]),
            "vsum": vs.astype(np.float32),
            "maskt": maskt,
        })

    nc = _get_nc()
    trace_dir = os.environ.get("KERNEL_TRACE_DIR")
    kwargs = {}
    if trace_dir:
        kwargs = dict(trace=True, trace_cores=[0], tmpdir=trace_dir)
    res = run_bass_kernel_spmd(nc, in_maps, list(range(8)), **kwargs)
    LAST_EXEC_NS = res.exec_time_ns

    const = (bv.astype(np.float64) @ Wo.astype(np.float64)
             + bo.astype(np.float64)).astype(np.float32)
    outp = np.empty((B, S, D), np.float32)
    for b in range(B):
        for sh in range(2):
            acc = (res.results[4 * b + sh]["out"]
                   + res.results[4 * b + 2 + sh]["out"] + const)
            outp[b, sh * SC:(sh + 1) * SC] = acc
    return outp



# revision 6
# speedup vs baseline: 1.2281x; 1.2281x over previous
"""Bass/Trainium2 kernel for nn_LocalAttention (banded attention, window 16).

Self-contained: takes full inputs, shards over 8 NeuronCores as
(batch, head-octet, seq-half), runs a banded-attention Bass kernel per core,
gathers on host.

Math: the reference zeroes out-of-band scores (not -inf) and softmaxes the
FULL row, so out-of-band entries contribute exp(0)=1.  With
em1 = (exp(s) - 1) * band_mask (exactly 0 off-band and on padded keys):
  Z_i   = sum_window(em1) + S
  num_i = sum_window(em1 * v) + sum_all(v)
so only a 144-wide banded computation per 128-query block is needed.
Scores are computed transposed ([keys, queries]) so em1 feeds the ctx matmul
as lhsT-free rhs directly.  All matmuls run in bf16 (4x the fp32 PE rate);
epilogues are fused single ops:
  Q:   (psum + bq) * 0.125            vector tensor_scalar (2-op)
  K:   Identity(psum + bk)            scalar activation, per-partition bias
  em1: (exp(s) - 1) * mask            scalar Exp + vector scalar_tensor_tensor
  Z:   += S via rank-1 ones matmul    (no vector add)
  ctx: (num + vsum) * (1/Z)           reciprocal + scalar_tensor_tensor
Head pairs are stacked on partitions 0:64 / 64:128 so attention epilogues run
at [128, 512] (half the instruction count).  Output is DMA'd straight from
PSUM.  bv/bo are folded on the host (softmax rows sum to 1); bk on padded
halo keys is cleared via per-core kpl/kpr multipliers.
"""
import os
import sys

for _p in ("/opt/trn_rl_repo",):
    if os.path.isdir(_p) and _p not in sys.path:
        sys.path.append(_p)

import numpy as np
import ml_dtypes

B, S, D = 2, 2048, 1024
H, HD = 16, 64
W = 16                    # band half-width 8
SC = 1024                 # seq chunk per core
HK = SC + W               # key halo chunk (1040)
HC = 512                  # head-dim columns per core (8 heads)
NBLK = SC // 128          # query blocks per head per core (8)
NH = HC // HD             # heads per core (8)

_CACHE = {}


def _build():
    import concourse.bacc as bacc
    import concourse.tile as tile
    from concourse import mybir

    f32 = mybir.dt.float32
    bf16 = mybir.dt.bfloat16
    Exp = mybir.ActivationFunctionType.Exp
    Ident = mybir.ActivationFunctionType.Identity
    add = mybir.AluOpType.add
    sub = mybir.AluOpType.subtract
    mult = mybir.AluOpType.mult

    nc = bacc.Bacc("TRN2", target_bir_lowering=False, debug=False, num_devices=8)

    xt = nc.dram_tensor("xt", [D, HK], bf16, kind="ExternalInput").ap()
    wq = nc.dram_tensor("wq", [D, HC], bf16, kind="ExternalInput").ap()
    wk = nc.dram_tensor("wk", [D, HC], bf16, kind="ExternalInput").ap()
    wv = nc.dram_tensor("wv", [D, HC], bf16, kind="ExternalInput").ap()
    wo = nc.dram_tensor("wo", [HC, D], bf16, kind="ExternalInput").ap()
    bq4 = nc.dram_tensor("bq4", [128, 4], f32, kind="ExternalInput").ap()
    bk4 = nc.dram_tensor("bk4", [128, 4], f32, kind="ExternalInput").ap()
    vsum = nc.dram_tensor("vsum", [128, 4], f32, kind="ExternalInput").ap()
    maskt = nc.dram_tensor("maskt", [128, 512], bf16, kind="ExternalInput").ap()
    kpl = nc.dram_tensor("kpl", [128, 1], f32, kind="ExternalInput").ap()
    kpr = nc.dram_tensor("kpr", [128, 1], f32, kind="ExternalInput").ap()
    out = nc.dram_tensor("out", [SC, D], f32, kind="ExternalOutput").ap()

    KD = D // 128     # 8 contraction tiles
    NVT = (HK + 127) // 128   # 9 V row tiles (last has 16 rows)

    with tile.TileContext(nc) as tc:
        with tc.tile_pool(name="stat", bufs=1) as stat, \
             tc.tile_pool(name="acts", bufs=1) as acts, \
             tc.tile_pool(name="sml", bufs=4) as sml, \
             tc.tile_pool(name="pmm", bufs=2, space="PSUM") as pmm, \
             tc.tile_pool(name="pst", bufs=2, space="PSUM") as pst, \
             tc.tile_pool(name="pcc", bufs=2, space="PSUM") as pcc, \
             tc.tile_pool(name="pzb", bufs=2, space="PSUM") as pzb:

            # ---- static inputs -> SBUF ----
            xt_sb = stat.tile([128, KD, HK], bf16)
            xt_r = xt.rearrange("(o p) f -> p o f", p=128)
            for k in range(KD):
                nc.sync.dma_start(xt_sb[:, k], xt_r[:, k])
            wq_sb = stat.tile([128, KD, HC], bf16)
            wk_sb = stat.tile([128, KD, HC], bf16)
            wv_sb = stat.tile([128, KD, HC], bf16)
            for w_sb, w_dr in ((wq_sb, wq), (wk_sb, wk), (wv_sb, wv)):
                nc.scalar.dma_start(w_sb[:], w_dr.rearrange("(o p) f -> p o f", p=128))
            wo_sb = stat.tile([128, HC // 128, D], bf16)
            nc.scalar.dma_start(wo_sb[:], wo.rearrange("(o p) f -> p o f", p=128))
            bq_sb = stat.tile([128, 4], f32)
            nc.sync.dma_start(bq_sb[:], bq4)
            bk_sb = stat.tile([128, 4], f32)
            nc.sync.dma_start(bk_sb[:], bk4)
            vsum_sb = stat.tile([128, 4], f32)
            nc.sync.dma_start(vsum_sb[:], vsum)
            mask_sb = stat.tile([128, 512], bf16)
            nc.sync.dma_start(mask_sb[:], maskt)
            kpl_sb = stat.tile([128, 1], f32)
            nc.sync.dma_start(kpl_sb[:], kpl)
            kpr_sb = stat.tile([128, 1], f32)
            nc.sync.dma_start(kpr_sb[:], kpr)

            onesm_sb = stat.tile([128, 128], bf16)
            nc.gpsimd.memset(onesm_sb[:], 1.0)
            sconst_sb = stat.tile([1, 512], bf16)
            nc.gpsimd.memset(sconst_sb[:], float(S))

            # ---- activations ----
            qt_sb = acts.tile([128, HC // 128, SC], bf16)    # Q^T * 0.125
            kt_sb = acts.tile([128, HC // 128, HK], bf16)    # K^T over halo keys
            vaug_sb = acts.tile([128, NVT, HC], bf16)        # V natural
            ctxt_sb = acts.tile([128, HC // 128, SC], bf16)  # ctx^T
            em_sb = acts.tile([128, 8, 512], bf16)           # em1 ring
            nc.gpsimd.memset(em_sb[:], 0.0)                  # keep junk regions finite

            # ---- projections (all bf16 matmuls, biases in epilogues) ----
            # Q^T = (x @ Wq + bq)^T * 0.125
            for m in range(HC // 128):
                for nch in range(SC // 512):
                    ps = pmm.tile([128, 512], f32, tag="mm")
                    for k in range(KD):
                        nc.tensor.matmul(
                            ps[:], wq_sb[:, k, m * 128:(m + 1) * 128],
                            xt_sb[:, k, 8 + nch * 512: 8 + (nch + 1) * 512],
                            start=(k == 0), stop=(k == KD - 1))
                    nc.vector.tensor_scalar(
                        qt_sb[:, m, nch * 512:(nch + 1) * 512], ps[:],
                        bq_sb[:, m:m + 1], 0.125, add, mult)

            # K^T over all HK halo keys (bias via scalar-engine epilogue)
            k_chunks = [(0, 512), (512, 512), (1024, HK - 1024)]
            for m in range(HC // 128):
                for (c0, cw) in k_chunks:
                    ps = pmm.tile([128, 512], f32, tag="mm")
                    for k in range(KD):
                        nc.tensor.matmul(
                            ps[:, :cw], wk_sb[:, k, m * 128:(m + 1) * 128],
                            xt_sb[:, k, c0:c0 + cw],
                            start=(k == 0), stop=(k == KD - 1))
                    nc.scalar.activation(kt_sb[:, m, c0:c0 + cw], ps[:, :cw],
                                         Ident, bias=bk_sb[:, m:m + 1])
                # clear bias on padded halo keys (kpl/kpr are 0 on edge cores)
                nc.vector.tensor_scalar_mul(kt_sb[:, m, 0:8],
                                            kt_sb[:, m, 0:8], kpl_sb[:, 0:1])
                nc.vector.tensor_scalar_mul(kt_sb[:, m, HK - 8:HK],
                                            kt_sb[:, m, HK - 8:HK],
                                            kpr_sb[:, 0:1])

            # V natural [HK, HC] (no bias on device; bv folded on host)
            for mt in range(NVT):
                rows = min(128, HK - mt * 128)
                ps = pmm.tile([128, 512], f32, tag="mm")
                for k in range(KD):
                    nc.tensor.matmul(
                        ps[:rows, :HC],
                        xt_sb[:, k, mt * 128: mt * 128 + rows],
                        wv_sb[:, k, :], start=(k == 0), stop=(k == KD - 1))
                nc.vector.tensor_copy(vaug_sb[:rows, mt, :], ps[:rows, :HC])

            # ---- banded attention, head pairs stacked on partitions ----
            # psT layout per (head, half): mains of blocks (j=0,1) at cols
            # 0:256, tails at 256:512 (partitions 0:16).
            for p in range(HC // 128):            # head pair = m-tile
                for tt in range(NBLK // 4):       # super-block of 4 q-blocks
                    for h01 in range(2):
                        hr = h01 * 64
                        for half in range(2):
                            psT = pst.tile([128, 512], f32, tag="st")
                            for j in range(2):
                                t = tt * 4 + half * 2 + j
                                nc.tensor.matmul(
                                    psT[:, j * 128:(j + 1) * 128],
                                    kt_sb[hr:hr + 64, p,
                                          t * 128: t * 128 + 128],
                                    qt_sb[hr:hr + 64, p,
                                          t * 128:(t + 1) * 128],
                                    start=True, stop=True)
                                nc.tensor.matmul(
                                    psT[0:16, 256 + j * 128: 256 + (j + 1) * 128],
                                    kt_sb[hr:hr + 64, p,
                                          t * 128 + 128: t * 128 + 144],
                                    qt_sb[hr:hr + 64, p,
                                          t * 128:(t + 1) * 128],
                                    start=True, stop=True)
                            slot = (tt % 2) * 4 + h01 * 2 + half
                            em = em_sb[:, slot]
                            nc.scalar.activation(em[:, 0:256], psT[:, 0:256], Exp)
                            nc.scalar.activation(em[0:16, 256:512],
                                                 psT[0:16, 256:512], Exp)
                            # em1 = (exp(s) - 1) * mask, junk regions -> 0
                            nc.vector.scalar_tensor_tensor(
                                em[:], em[:], 1.0, mask_sb[:], sub, mult)

                    ps_c = pcc.tile([128, 512], f32, tag="cc")
                    ps_z = pzb.tile([128, 512], f32, tag="zb")
                    # Z starts at S everywhere (rank-1 broadcast matmul)
                    nc.tensor.matmul(ps_z[:], onesm_sb[0:1, 0:128],
                                     sconst_sb[0:1, :], start=True, stop=False,
                                     skip_group_check=True)
                    for h01 in range(2):
                        h = 2 * p + h01
                        for q in range(4):
                            t = tt * 4 + q
                            slot = (tt % 2) * 4 + h01 * 2 + (q // 2)
                            em = em_sb[:, slot]
                            j = q % 2
                            rc = slice(h01 * 64, h01 * 64 + 64)
                            qc = slice(q * 128, (q + 1) * 128)
                            nc.tensor.matmul(
                                ps_c[rc, qc],
                                vaug_sb[:, t, h * HD:(h + 1) * HD],
                                em[:, j * 128:(j + 1) * 128],
                                start=True, stop=False)
                            nc.tensor.matmul(
                                ps_c[rc, qc],
                                vaug_sb[0:16, t + 1, h * HD:(h + 1) * HD],
                                em[0:16, 256 + j * 128: 256 + (j + 1) * 128],
                                start=False, stop=True)
                            nc.tensor.matmul(
                                ps_z[rc, qc], onesm_sb[:, 0:64],
                                em[:, j * 128:(j + 1) * 128],
                                start=False, stop=False,
                                skip_group_check=True)
                            nc.tensor.matmul(
                                ps_z[rc, qc], onesm_sb[0:16, 0:64],
                                em[0:16, 256 + j * 128: 256 + (j + 1) * 128],
                                start=False,
                                stop=(h01 == 1 and q == 3),
                                skip_group_check=True)
                    # ctx = (num + vsum) * 1/(z + S)
                    rzb = sml.tile([128, 512], f32, tag="rz")
                    nc.vector.reciprocal_approx_fast(rzb[:], ps_z[:])
                    nc.vector.scalar_tensor_tensor(
                        ctxt_sb[:, p, tt * 512:(tt + 1) * 512],
                        ps_c[:], vsum_sb[:, p:p + 1], rzb[:], add, mult)

            # ---- out projection ----
            for st in range(SC // 128):
                for nch in range(D // 512):
                    ps = pmm.tile([128, 512], f32, tag="mm")
                    for kt in range(HC // 128):
                        nc.tensor.matmul(
                            ps[:], ctxt_sb[:, kt, st * 128:(st + 1) * 128],
                            wo_sb[:, kt, nch * 512:(nch + 1) * 512],
                            start=(kt == 0), stop=(kt == HC // 128 - 1))
                    o_sb = sml.tile([128, 512], f32, tag="ob")
                    if (st * 2 + nch) % 2 == 0:
                        nc.vector.tensor_copy(o_sb[:], ps[:])
                    else:
                        nc.scalar.activation(o_sb[:], ps[:],
                                             mybir.ActivationFunctionType.Copy)
                    nc.sync.dma_start(
                        out[st * 128:(st + 1) * 128, nch * 512:(nch + 1) * 512],
                        o_sb[:])

    nc.compile()
    return nc


def _get_nc():
    if "nc" not in _CACHE:
        _CACHE["nc"] = _build()
    return _CACHE["nc"]


LAST_EXEC_NS = None


def _band_maskt():
    """[128, 512] bf16: mains at cols 0:256, tails at 256:512 (rows 0:16)."""
    m = np.zeros((128, 512), np.float32)
    r = np.arange(128)[:, None]
    c = np.arange(128)[None, :]
    main = ((c <= r) & (r <= c + W)).astype(np.float32)   # keys 0..127
    r2 = np.arange(16)[:, None]
    tail = (c >= 112 + r2).astype(np.float32)             # keys 128..143
    for j in (0, 1):
        m[:, j * 128:(j + 1) * 128] = main
        m[:16, 256 + j * 128: 256 + (j + 1) * 128] = tail
    return m.astype(ml_dtypes.bfloat16)


def kernel(hidden_states, Wq, bq, Wk, bk, Wv, bv, Wo, bo):
    global LAST_EXEC_NS
    from concourse.bass_utils import run_bass_kernel_spmd

    bf = ml_dtypes.bfloat16
    hs = np.asarray(hidden_states, dtype=np.float32)
    Wq, Wk, Wv, Wo = (np.asarray(a, dtype=np.float32) for a in (Wq, Wk, Wv, Wo))
    bq, bk, bv, bo = (np.asarray(a, dtype=np.float32) for a in (bq, bk, bv, bo))

    xpad = np.zeros((B, S + W, D), np.float32)
    xpad[:, 8:8 + S] = hs
    xT = np.ascontiguousarray(xpad.transpose(0, 2, 1))  # [B, D, S+W]

    maskt = _band_maskt()
    ones_col = np.ones((128, 1), np.float32)
    zero_col = np.zeros((128, 1), np.float32)

    in_maps = []
    for core in range(8):
        b, hg, sh = core // 4, (core // 2) % 2, core % 2
        cols = slice(hg * HC, (hg + 1) * HC)
        vs = xpad[b].sum(0, dtype=np.float64) @ Wv[:, cols].astype(np.float64)
        in_maps.append({
            "xt": xT[b][:, sh * SC: sh * SC + HK].astype(bf),
            "wq": Wq[:, cols].astype(bf),
            "wk": Wk[:, cols].astype(bf),
            "wv": Wv[:, cols].astype(bf),
            "wo": np.ascontiguousarray(Wo[cols, :]).astype(bf),
            "bq4": np.ascontiguousarray(bq[cols].reshape(4, 128).T),
            "bk4": np.ascontiguousarray(bk[cols].reshape(4, 128).T),
            "vsum": np.ascontiguousarray(
                vs.astype(np.float32).reshape(4, 128).T),
            "maskt": maskt,
            "kpl": zero_col if sh == 0 else ones_col,
            "kpr": zero_col if sh == 1 else ones_col,
        })

    nc = _get_nc()
    trace_dir = os.environ.get("KERNEL_TRACE_DIR")
    kwargs = {}
    if trace_dir:
        kwargs = dict(trace=True, trace_cores=[0], tmpdir=trace_dir)
    res = run_bass_kernel_spmd(nc, in_maps, list(range(8)), **kwargs)
    LAST_EXEC_NS = res.exec_time_ns

    const = (bv.astype(np.float64) @ Wo.astype(np.float64)
             + bo.astype(np.float64)).astype(np.float32)
    outp = np.empty((B, S, D), np.float32)
    for b in range(B):
        for sh in range(2):
            acc = (res.results[4 * b + sh]["out"]
                   + res.results[4 * b + 2 + sh]["out"] + const)
            outp[b, sh * SC:(sh + 1) * SC] = acc
    return outp


# revision 9
# speedup vs baseline: 2.0285x; 1.6518x over previous
"""Bass/Trainium2 kernel for nn_LocalAttention (banded attention, window 16).

Self-contained: takes full inputs, shards over 8 NeuronCores as
(batch, head-octet, seq-half), runs a banded-attention Bass kernel per core,
gathers on host.

Math: the reference zeroes out-of-band scores (not -inf) and softmaxes the
FULL row, so out-of-band entries contribute exp(0)=1.  With
em1 = (exp(s) - 1) * band_mask (exactly 0 off-band and on padded keys):
  Z_i   = sum_window(em1) + S
  num_i = sum_window(em1 * v) + sum_all(v)
so only a banded computation per query block is needed.

Query blocks are 112 wide so each block's key window is 112+16 = 128 keys:
scores / ctx are then ONE matmul per block (no 16-key tail matmuls), the
whole [128 keys, 448 queries] score tile is maskable in a single fused op,
and Z is one N=448 matmul per head.  A 16-query rump block (queries
1008:1023, 32 keys) completes the sequence.  V is projected into 128-row
tiles at a 112 stride so each block's key window is partition-aligned.

All matmuls are bf16.  Epilogues are single fused ops:
  Q:   (psum + bq) * 0.125            vector tensor_scalar (2-op)
  K:   Identity(psum + bk)            scalar activation, per-partition bias
  em1: (exp(s) - 1) * mask            scalar Exp + vector scalar_tensor_tensor
  Z:   += S via rank-1 ones matmul
  ctx: (num + vsum) * (1/Z)           reciprocal_approx_fast + fused op
Head pairs are stacked on partitions 0:64 / 64:128.  Out-projection tiles are
emitted as soon as their ctxt columns are ready, overlapping the output DMA
with attention.  bv/bo are folded on the host (softmax rows sum to 1); bk on
padded halo keys is cleared via per-core kpl/kpr multipliers.
"""
import os
import sys

for _p in ("/opt/trn_rl_repo",):
    if os.path.isdir(_p) and _p not in sys.path:
        sys.path.append(_p)

import numpy as np
import ml_dtypes

B, S, D = 2, 2048, 1024
H, HD = 16, 64
W = 16                    # band half-width 8
SC = 1024                 # seq chunk per core
HK = SC + W               # key halo chunk (1040)
HC = 512                  # head-dim columns per core (8 heads)
NH = HC // HD             # heads per core (8)
QB = 112                  # queries per block (window = QB + W = 128 keys)
NFB = 9                   # full blocks per head (9*112 = 1008)
RQ = SC - NFB * QB        # rump queries (16)

_CACHE = {}


def _build():
    import concourse.bacc as bacc
    import concourse.tile as tile
    from concourse import mybir

    f32 = mybir.dt.float32
    bf16 = mybir.dt.bfloat16
    Exp = mybir.ActivationFunctionType.Exp
    Ident = mybir.ActivationFunctionType.Identity
    Copy = mybir.ActivationFunctionType.Copy
    add = mybir.AluOpType.add
    sub = mybir.AluOpType.subtract
    mult = mybir.AluOpType.mult

    nc = bacc.Bacc("TRN2", target_bir_lowering=False, debug=False, num_devices=8)

    xt = nc.dram_tensor("xt", [D, HK], bf16, kind="ExternalInput").ap()
    wq = nc.dram_tensor("wq", [D, HC], bf16, kind="ExternalInput").ap()
    wk = nc.dram_tensor("wk", [D, HC], bf16, kind="ExternalInput").ap()
    wv = nc.dram_tensor("wv", [D, HC], bf16, kind="ExternalInput").ap()
    wo = nc.dram_tensor("wo", [HC, D], bf16, kind="ExternalInput").ap()
    bq4 = nc.dram_tensor("bq4", [128, 4], f32, kind="ExternalInput").ap()
    bk4 = nc.dram_tensor("bk4", [128, 4], f32, kind="ExternalInput").ap()
    vsum = nc.dram_tensor("vsum", [128, 4], f32, kind="ExternalInput").ap()
    maskt = nc.dram_tensor("maskt", [128, 576], bf16, kind="ExternalInput").ap()
    kpl = nc.dram_tensor("kpl", [128, 1], f32, kind="ExternalInput").ap()
    kpr = nc.dram_tensor("kpr", [128, 1], f32, kind="ExternalInput").ap()
    out = nc.dram_tensor("out", [SC, D], f32, kind="ExternalOutput").ap()

    KD = D // 128     # 8 contraction tiles
    NVT = NFB + 1     # 9 full V tiles at 112 stride + 1 rump tile

    with tile.TileContext(nc) as tc:
        with tc.tile_pool(name="stat", bufs=1) as stat, \
             tc.tile_pool(name="acts", bufs=1) as acts, \
             tc.tile_pool(name="sml", bufs=4) as sml, \
             tc.tile_pool(name="pmm", bufs=2, space="PSUM") as pmm, \
             tc.tile_pool(name="pst", bufs=2, space="PSUM") as pst, \
             tc.tile_pool(name="pcc", bufs=2, space="PSUM") as pcc, \
             tc.tile_pool(name="pzb", bufs=2, space="PSUM") as pzb:

            # ---- static inputs -> SBUF (two DMA queues, compute-order) ----
            xt_sb = stat.tile([128, KD, HK], bf16)
            xt_r = xt.rearrange("(o p) f -> p o f", p=128)
            wq_sb = stat.tile([128, KD, HC], bf16)
            wk_sb = stat.tile([128, KD, HC], bf16)
            wv_sb = stat.tile([128, KD, HC], bf16)
            wo_sb = stat.tile([128, HC // 128, D], bf16)
            nc.sync.dma_start(xt_sb[:, :, 0:520], xt_r[:, :, 0:520])
            nc.scalar.dma_start(wq_sb[:], wq.rearrange("(o p) f -> p o f", p=128))
            nc.scalar.dma_start(xt_sb[:, :, 520:HK], xt_r[:, :, 520:HK])
            nc.scalar.dma_start(wk_sb[:], wk.rearrange("(o p) f -> p o f", p=128))
            nc.scalar.dma_start(wv_sb[:], wv.rearrange("(o p) f -> p o f", p=128))
            nc.scalar.dma_start(wo_sb[:], wo.rearrange("(o p) f -> p o f", p=128))
            bq_sb = stat.tile([128, 4], f32)
            nc.sync.dma_start(bq_sb[:], bq4)
            bk_sb = stat.tile([128, 4], f32)
            nc.sync.dma_start(bk_sb[:], bk4)
            vsum_sb = stat.tile([128, 4], f32)
            nc.sync.dma_start(vsum_sb[:], vsum)
            mask_sb = stat.tile([128, 576], bf16)
            nc.sync.dma_start(mask_sb[:], maskt)
            kpl_sb = stat.tile([128, 1], f32)
            nc.sync.dma_start(kpl_sb[:], kpl)
            kpr_sb = stat.tile([128, 1], f32)
            nc.sync.dma_start(kpr_sb[:], kpr)

            onesm_sb = stat.tile([128, 128], bf16)
            nc.gpsimd.memset(onesm_sb[:], 1.0)
            sconst_sb = stat.tile([1, 448], bf16)
            nc.gpsimd.memset(sconst_sb[:], float(S))

            # ---- activations ----
            qt_sb = acts.tile([128, HC // 128, SC], bf16)    # Q^T * 0.125
            kt_sb = acts.tile([128, HC // 128, HK], bf16)    # K^T over halo keys
            vaug_sb = acts.tile([128, NVT, HC], bf16)        # V, 112-stride tiles
            ctxt_sb = acts.tile([128, HC // 128, SC], bf16)  # ctx^T
            em_sb = acts.tile([128, 4, 448], bf16)           # em1 ring
            nc.gpsimd.memset(em_sb[:], 0.0)                  # keep junk finite

            # ---- projections (all bf16 matmuls, biases in epilogues) ----
            # Q^T = (x @ Wq + bq)^T * 0.125
            for m in range(HC // 128):
                for nch in range(SC // 512):
                    ps = pmm.tile([128, 512], f32, tag="mm")
                    for k in range(KD):
                        nc.tensor.matmul(
                            ps[:], wq_sb[:, k, m * 128:(m + 1) * 128],
                            xt_sb[:, k, 8 + nch * 512: 8 + (nch + 1) * 512],
                            start=(k == 0), stop=(k == KD - 1))
                    nc.vector.tensor_scalar(
                        qt_sb[:, m, nch * 512:(nch + 1) * 512], ps[:],
                        bq_sb[:, m:m + 1], 0.125, add, mult)

            # K^T over all HK halo keys (bias via scalar-engine epilogue)
            k_chunks = [(0, 512), (512, 512), (1024, HK - 1024)]
            for m in range(HC // 128):
                for (c0, cw) in k_chunks:
                    ps = pmm.tile([128, 512], f32, tag="mm")
                    for k in range(KD):
                        nc.tensor.matmul(
                            ps[:, :cw], wk_sb[:, k, m * 128:(m + 1) * 128],
                            xt_sb[:, k, c0:c0 + cw],
                            start=(k == 0), stop=(k == KD - 1))
                    nc.scalar.activation(kt_sb[:, m, c0:c0 + cw], ps[:, :cw],
                                         Ident, bias=bk_sb[:, m:m + 1])
                # clear bias on padded halo keys (kpl/kpr are 0 on edge cores)
                nc.vector.tensor_scalar_mul(kt_sb[:, m, 0:8],
                                            kt_sb[:, m, 0:8], kpl_sb[:, 0:1])
                nc.vector.tensor_scalar_mul(kt_sb[:, m, HK - 8:HK],
                                            kt_sb[:, m, HK - 8:HK],
                                            kpr_sb[:, 0:1])

            # V tiles: tile t = keys [t*112, t*112+128) (ctx lhsT windows);
            # rump tile 9 = keys [1000, 1040).
            for mt in range(NVT):
                off = mt * QB if mt < NFB else HK - 32
                rows = 128 if mt < NFB else 32
                ps = pmm.tile([128, 512], f32, tag="mm")
                for k in range(KD):
                    nc.tensor.matmul(
                        ps[:rows, :HC],
                        xt_sb[:, k, off: off + rows],
                        wv_sb[:, k, :], start=(k == 0), stop=(k == KD - 1))
                nc.vector.tensor_copy(vaug_sb[:rows, mt, :], ps[:rows, :HC])

            # ---- out-projection emitter (interleaved with attention) ----
            def emit_out(st):
                for nch in range(D // 512):
                    ps = pmm.tile([128, 512], f32, tag="mm")
                    for kt in range(HC // 128):
                        nc.tensor.matmul(
                            ps[:], ctxt_sb[:, kt, st * 128:(st + 1) * 128],
                            wo_sb[:, kt, nch * 512:(nch + 1) * 512],
                            start=(kt == 0), stop=(kt == HC // 128 - 1))
                    o_sb = sml.tile([128, 512], f32, tag="ob")
                    if (st * 2 + nch) % 2 == 0:
                        nc.vector.tensor_copy(o_sb[:], ps[:])
                    else:
                        nc.scalar.activation(o_sb[:], ps[:], Copy)
                    nc.sync.dma_start(
                        out[st * 128:(st + 1) * 128,
                            nch * 512:(nch + 1) * 512], o_sb[:])

            # ---- banded attention ----
            # Supers tt=0,1: 4 full blocks each -> [128 keys, 448 queries]
            # tiles; super tt=2: block 8 + 16-query rump -> [128, 128].
            # Head pairs stacked on partitions (head 2p+h01 at h01*64).
            sidx = 0
            for tt in range(3):
                nblk = 4 if tt < 2 else 1
                cw = 448 if tt < 2 else 128
                moff = 0 if tt < 2 else 448
                for p in range(HC // 128):
                    ems = []
                    for h01 in range(2):
                        hr = h01 * 64
                        psT = pst.tile([128, 448], f32, tag="st")
                        for i in range(nblk):
                            t = tt * 4 + i
                            nc.tensor.matmul(
                                psT[:, i * QB:(i + 1) * QB],
                                kt_sb[hr:hr + 64, p, t * QB: t * QB + 128],
                                qt_sb[hr:hr + 64, p, t * QB:(t + 1) * QB],
                                start=True, stop=True)
                        if tt == 2:   # rump: queries 1008:1024, keys 1008:1040
                            nc.tensor.matmul(
                                psT[0:32, QB:128],
                                kt_sb[hr:hr + 64, p, HK - 32:HK],
                                qt_sb[hr:hr + 64, p, SC - RQ:SC],
                                start=True, stop=True)
                        slot = (sidx % 2) * 2 + h01
                        em = em_sb[:, slot]
                        if tt < 2:
                            nc.scalar.activation(em[:, 0:448], psT[:, 0:448],
                                                 Exp)
                        else:
                            nc.scalar.activation(em[:, 0:QB], psT[:, 0:QB],
                                                 Exp)
                            nc.scalar.activation(em[0:32, QB:128],
                                                 psT[0:32, QB:128], Exp)
                        # em1 = (exp(s) - 1) * mask, junk regions -> 0
                        nc.vector.scalar_tensor_tensor(
                            em[:, 0:cw], em[:, 0:cw], 1.0,
                            mask_sb[:, moff:moff + cw], sub, mult)
                        ems.append(em)

                    ps_c = pcc.tile([128, 448], f32, tag="cc")
                    ps_z = pzb.tile([128, 448], f32, tag="zb")
                    # Z starts at S everywhere (rank-1 broadcast matmul)
                    nc.tensor.matmul(ps_z[:, 0:cw], onesm_sb[0:1, 0:128],
                                     sconst_sb[0:1, 0:cw], start=True,
                                     stop=False, skip_group_check=True)
                    for h01 in range(2):
                        h = 2 * p + h01
                        em = ems[h01]
                        rc = slice(h01 * 64, h01 * 64 + 64)
                        for i in range(nblk):
                            t = tt * 4 + i
                            nc.tensor.matmul(
                                ps_c[rc, i * QB:(i + 1) * QB],
                                vaug_sb[:, t, h * HD:(h + 1) * HD],
                                em[:, i * QB:(i + 1) * QB],
                                start=True, stop=True)
                        if tt == 2:
                            nc.tensor.matmul(
                                ps_c[rc, QB:128],
                                vaug_sb[0:32, NFB, h * HD:(h + 1) * HD],
                                em[0:32, QB:128], start=True, stop=True)
                        nc.tensor.matmul(
                            ps_z[rc, 0:cw], onesm_sb[:, 0:64], em[:, 0:cw],
                            start=False, stop=(h01 == 1),
                            skip_group_check=True)
                    # ctx = (num + vsum) * 1/(z + S)
                    rzb = sml.tile([128, 448], f32, tag="rz")
                    nc.vector.reciprocal_approx_fast(rzb[:, 0:cw],
                                                     ps_z[:, 0:cw])
                    nc.vector.scalar_tensor_tensor(
                        ctxt_sb[:, p, tt * 448: tt * 448 + cw],
                        ps_c[:, 0:cw], vsum_sb[:, p:p + 1], rzb[:, 0:cw],
                        add, mult)
                    sidx += 1
                # emit out-proj tiles whose ctxt columns are now complete
                if tt == 0:
                    for st in (0, 1, 2):
                        emit_out(st)
                elif tt == 1:
                    for st in (3, 4, 5, 6):
                        emit_out(st)
                else:
                    emit_out(7)

    nc.compile()
    return nc


def _get_nc():
    if "nc" not in _CACHE:
        _CACHE["nc"] = _build()
    return _CACHE["nc"]


LAST_EXEC_NS = None


def _band_maskt():
    """[128, 576] bf16: cols 0:448 = four 112-query main masks; cols 448:576
    = super-2 mask (112-query main + 16-query/32-key rump)."""
    m = np.zeros((128, 576), np.float32)
    k = np.arange(128)[:, None]
    q = np.arange(QB)[None, :]
    main = ((q <= k) & (k <= q + W)).astype(np.float32)
    for j in range(4):
        m[:, j * QB:(j + 1) * QB] = main
    m[:, 448:448 + QB] = main
    kr = np.arange(32)[:, None]
    qr = np.arange(RQ)[None, :]
    m[:32, 448 + QB:448 + 128] = ((qr <= kr) & (kr <= qr + W)).astype(np.float32)
    return m.astype(ml_dtypes.bfloat16)


def kernel(hidden_states, Wq, bq, Wk, bk, Wv, bv, Wo, bo):
    global LAST_EXEC_NS
    from concourse.bass_utils import run_bass_kernel_spmd

    bf = ml_dtypes.bfloat16
    hs = np.asarray(hidden_states, dtype=np.float32)
    Wq, Wk, Wv, Wo = (np.asarray(a, dtype=np.float32) for a in (Wq, Wk, Wv, Wo))
    bq, bk, bv, bo = (np.asarray(a, dtype=np.float32) for a in (bq, bk, bv, bo))

    xpad = np.zeros((B, S + W, D), np.float32)
    xpad[:, 8:8 + S] = hs
    xT = np.ascontiguousarray(xpad.transpose(0, 2, 1))  # [B, D, S+W]

    maskt = _band_maskt()
    ones_col = np.ones((128, 1), np.float32)
    zero_col = np.zeros((128, 1), np.float32)

    in_maps = []
    for core in range(8):
        b, hg, sh = core // 4, (core // 2) % 2, core % 2
        cols = slice(hg * HC, (hg + 1) * HC)
        vs = xpad[b].sum(0, dtype=np.float64) @ Wv[:, cols].astype(np.float64)
        in_maps.append({
            "xt": xT[b][:, sh * SC: sh * SC + HK].astype(bf),
            "wq": Wq[:, cols].astype(bf),
            "wk": Wk[:, cols].astype(bf),
            "wv": Wv[:, cols].astype(bf),
            "wo": np.ascontiguousarray(Wo[cols, :]).astype(bf),
            "bq4": np.ascontiguousarray(bq[cols].reshape(4, 128).T),
            "bk4": np.ascontiguousarray(bk[cols].reshape(4, 128).T),
            "vsum": np.ascontiguousarray(
                vs.astype(np.float32).reshape(4, 128).T),
            "maskt": maskt,
            "kpl": zero_col if sh == 0 else ones_col,
            "kpr": zero_col if sh == 1 else ones_col,
        })

    nc = _get_nc()
    trace_dir = os.environ.get("KERNEL_TRACE_DIR")
    kwargs = {}
    if trace_dir:
        kwargs = dict(trace=True, trace_cores=[0], tmpdir=trace_dir)
    res = run_bass_kernel_spmd(nc, in_maps, list(range(8)), **kwargs)
    LAST_EXEC_NS = res.exec_time_ns

    const = (bv.astype(np.float64) @ Wo.astype(np.float64)
             + bo.astype(np.float64)).astype(np.float32)
    outp = np.empty((B, S, D), np.float32)
    for b in range(B):
        for sh in range(2):
            acc = (res.results[4 * b + sh]["out"]
                   + res.results[4 * b + 2 + sh]["out"] + const)
            outp[b, sh * SC:(sh + 1) * SC] = acc
    return outp
